# revision 7
# baseline (speedup 1.0000x reference)
"""Bass/Trainium2 kernel for the BarlowTwins-style cross-entropy loss.

Reference (per batch b of 8):
    logits = z1[b].T @ z2[b] / T            (2048 x 2048, K=256, T=1.0)
    logp   = log_softmax(logits, axis=0)    (softmax over first axis n)
    loss   = -mean_b,m logp[m, m]

Sharding: pure data parallel over the batch axis b -> one batch element per
NeuronCore (8 cores).  Each core computes logitsT[m, n] = sum_s z2[s,m]*z1[s,n]
so the softmax reduction runs along the free axis.

Per 128-row chunk of logitsT, processed as two [128,1024] halves so four
2-bank PSUM tiles keep the PE / DVE / ACT pipeline full:
    nmx_h[m] = -max_{n in h} logitsT[m, n]     (DVE tensor_reduce, negated)
    se_h[m]  = sum_{n in h} exp(logitsT[m,n] + nmx_h[m])   (ACT exp + accum)
Host merge: M = max over halves; se = sum_h se_h * e^(mx_h - M);
            diag[m] = sum_s z1[s,m]*z2[s,m] computed on host in f32 (0.02% of
            the problem FLOPs; the 17 GFLOP logits/softmax runs on device);
            loss = -mean(diag - M - log(se)).

Inputs are converted to bf16 on the host (halves DMA traffic; PE runs bf16 at
full rate with f32 PSUM accumulation; loss error vs f32 reference ~1e-5).
"""

import numpy as np
import ml_dtypes

import concourse.bass as bass
import concourse.tile as tile
from concourse import bacc, mybir
from concourse.bass_utils import run_bass_kernel_spmd

B = 8          # batch (one element per core)
S = 256        # contraction dim
N = 2048       # feature dim (n and m)
P = 128        # SBUF partitions
KC = S // P    # 2 contraction chunks
MC = N // P    # 16 row chunks of logitsT
H = N // 2     # half width (one PSUM tile)

_CACHE = {}


def _build():
    if "nc" in _CACHE:
        return _CACHE["nc"]

    f32 = mybir.dt.float32
    bf16 = mybir.dt.bfloat16

    nc = bacc.Bacc("TRN2", target_bir_lowering=False, debug=False)
    z1 = nc.dram_tensor("z1", [S, N], bf16, kind="ExternalInput").ap()
    z2 = nc.dram_tensor("z2", [S, N], bf16, kind="ExternalInput").ap()
    # out[:, 0:32]  = negated half maxes  (col 2m+h)
    # out[:, 32:64] = half exp-sums       (col 32+2m+h)
    out_d = nc.dram_tensor("out", [P, 64], f32, kind="ExternalOutput").ap()

    z1r = z1.rearrange("(k p) n -> k p n", p=P)
    z2r = z2.rearrange("(k p) n -> k p n", p=P)

    with tile.TileContext(nc) as tc:
        with (
            tc.tile_pool(name="const", bufs=1) as cpool,
            tc.tile_pool(name="zb", bufs=1) as zpool,
            tc.tile_pool(name="psum", bufs=4, space="PSUM") as ppool,
            tc.tile_pool(name="trash", bufs=3) as tpool,
            tc.tile_pool(name="mx", bufs=8) as mpool,
            tc.tile_pool(name="se", bufs=8) as spool,
        ):
            # ACT exp-table preload, overlapped with the input DMAs.
            dummy = cpool.tile([1, 1], f32, tag="dummy")
            nc.gpsimd.memset(dummy[:], 0.0)
            nc.scalar.activation(
                dummy[:], dummy[:], mybir.ActivationFunctionType.Exp, bias=0.0
            )

            osb = cpool.tile([P, 64], f32, tag="osb")

            z1b = [
                zpool.tile([P, N], bf16, name=f"z1b{k}", tag=f"z1b{k}")
                for k in range(KC)
            ]
            z2b = [
                zpool.tile([P, N], bf16, name=f"z2b{k}", tag=f"z2b{k}")
                for k in range(KC)
            ]
            # Lean DMA plan: first-needed slices first, spread over both
            # DGE-capable queues so dispatch overlaps.
            nc.sync.dma_start(z2b[0][:, 0:P], z2r[0][:, 0:P])
            nc.scalar.dma_start(z2b[1][:, 0:P], z2r[1][:, 0:P])
            nc.sync.dma_start(z1b[0][:, 0:512], z1r[0][:, 0:512])
            nc.scalar.dma_start(z1b[1][:, 0:512], z1r[1][:, 0:512])
            nc.sync.dma_start(z1b[0][:, 512:H], z1r[0][:, 512:H])
            nc.scalar.dma_start(z1b[1][:, 512:H], z1r[1][:, 512:H])
            nc.sync.dma_start(z1b[0][:, H:N], z1r[0][:, H:N])
            nc.scalar.dma_start(z1b[1][:, H:N], z1r[1][:, H:N])
            nc.sync.dma_start(z2b[0][:, P:N], z2r[0][:, P:N])
            nc.scalar.dma_start(z2b[1][:, P:N], z2r[1][:, P:N])

            # PE warm-up during the input-DMA wait: the HAM clock gate keeps
            # the PE at 1.2 GHz until it has been busy ~3.4us.  A burst of
            # junk matmuls on zeroed SBUF (no DMA dependency) starts that
            # clock early so the first real chunks run at 2.4 GHz.
            warm = cpool.tile([P, 512], bf16, tag="warm")
            nc.gpsimd.memset(warm[:], 0.0)
            jpsum = ppool.tile([P, H], f32, tag="psum")
            for _ in range(6):
                nc.tensor.matmul(
                    jpsum[:, 0:512],
                    lhsT=warm[:, 0:P],
                    rhs=warm[:],
                    start=True,
                    stop=True,
                )

            for m in range(MC):
                ms = slice(m * P, (m + 1) * P)
                for h in range(2):
                    hbase = h * H
                    psum = ppool.tile([P, H], f32, tag="psum")
                    for k in range(KC):
                        for nb in range(2):
                            ns = slice(hbase + nb * 512, hbase + (nb + 1) * 512)
                            nc.tensor.matmul(
                                psum[:, nb * 512 : (nb + 1) * 512],
                                lhsT=z2b[k][:, ms],
                                rhs=z1b[k][:, ns],
                                start=(k == 0),
                                stop=(k == KC - 1),
                            )
                    j = 2 * m + h
                    # negated half-row max -> bias for the exp.  Small
                    # rotating tiles decouple the hot DVE->ACT chain from the
                    # staging tile (GpSimd copies into osb off the hot path).
                    mx_t = mpool.tile([P, 1], f32, tag="mx")
                    se_t = spool.tile([P, 1], f32, tag="se")
                    nc.vector.tensor_reduce(
                        mx_t[:],
                        psum[:],
                        axis=mybir.AxisListType.X,
                        op=mybir.AluOpType.max,
                        negate=True,
                    )
                    # exp(logitsT - halfmax), accumulated along the half row;
                    # exp values written back in place (discarded).
                    nc.scalar.activation(
                        psum[:],
                        psum[:],
                        mybir.ActivationFunctionType.Exp,
                        bias=mx_t[:],
                        scale=1.0,
                        accum_out=se_t[:],
                    )
                    nc.gpsimd.tensor_copy(osb[:, j : j + 1], mx_t[:])
                    nc.gpsimd.tensor_copy(osb[:, 32 + j : 33 + j], se_t[:])
                if m == 13:
                    # Overlap most of the output DMA with the last chunks.
                    nc.sync.dma_start(out_d[:, 0:28], osb[:, 0:28])
                    nc.sync.dma_start(out_d[:, 32:60], osb[:, 32:60])

            nc.sync.dma_start(out_d[:, 28:32], osb[:, 28:32])
            nc.sync.dma_start(out_d[:, 60:64], osb[:, 60:64])

    nc.compile()
    _CACHE["nc"] = nc
    return nc


def _run(z1, z2, **spmd_kwargs):
    """Shard over batch, run on 8 cores, return (loss, BassKernelResults)."""
    nc = _build()
    z1 = np.ascontiguousarray(z1, dtype=np.float32)
    z2 = np.ascontiguousarray(z2, dtype=np.float32)
    in_maps = [
        {
            "z1": np.ascontiguousarray(z1[b].astype(ml_dtypes.bfloat16)),
            "z2": np.ascontiguousarray(z2[b].astype(ml_dtypes.bfloat16)),
        }
        for b in range(B)
    ]
    res = run_bass_kernel_spmd(nc, in_maps, core_ids=list(range(B)), **spmd_kwargs)

    # diag[b, m] = sum_s z1[b,s,m] * z2[b,s,m] in f32 on host (tiny vs the
    # 17 GFLOP device part).
    dg = np.einsum("bsm,bsm->bm", z1, z2, dtype=np.float64)

    total = 0.0
    for b in range(B):
        o = res.results[b]["out"].astype(np.float64)   # [P, 64]
        nmx = o[:, 0:32]                               # [P, 2m+h] negated half max
        se2 = o[:, 32:64]
        ma = -nmx[:, 0::2]                             # [P, MC] left-half max
        mb = -nmx[:, 1::2]
        sea = se2[:, 0::2]
        seb = se2[:, 1::2]
        M = np.maximum(ma, mb)
        se = sea * np.exp(ma - M) + seb * np.exp(mb - M)
        logZ = (M + np.log(se)).T.reshape(N)           # row index = m*P + p
        total += np.sum(dg[b] - logZ)
    loss = -total / (B * N)
    return np.asarray(loss, dtype=np.float32), res


def kernel(z1, z2):
    loss, _ = _run(z1, z2)
    return loss


# revision 8
# speedup vs baseline: 1.0886x; 1.0886x over previous
"""Bass/Trainium2 kernel for the BarlowTwins-style cross-entropy loss.

Reference (per batch b of 8):
    logits = z1[b].T @ z2[b] / T            (2048 x 2048, K=256, T=1.0)
    logp   = log_softmax(logits, axis=0)    (softmax over first axis n)
    loss   = -mean_b,m logp[m, m]

Sharding: pure data parallel over the batch axis b -> one batch element per
NeuronCore (8 cores).  Each core computes logitsT[m, n] = sum_s z2[s,m]*z1[s,n]
so the softmax reduction runs along the free axis.

Per 128-row chunk of logitsT, processed as two [128,1024] halves so four
2-bank PSUM tiles keep the PE / DVE / ACT pipeline full:
    nmx_h[m] = -max_{n in h} logitsT[m, n]     (DVE tensor_reduce, negated)
    se_h[m]  = sum_{n in h} exp(logitsT[m,n] + nmx_h[m])   (ACT exp + accum)
Host merge: M = max over halves; se = sum_h se_h * e^(mx_h - M);
            diag[m] = sum_s z1[s,m]*z2[s,m] computed on host in f32 (0.02% of
            the problem FLOPs; the 17 GFLOP logits/softmax runs on device);
            loss = -mean(diag - M - log(se)).

Inputs are converted to bf16 on the host (halves DMA traffic; PE runs bf16 at
full rate with f32 PSUM accumulation; loss error vs f32 reference ~1e-5).
"""

import numpy as np
import ml_dtypes

import concourse.bass as bass
import concourse.tile as tile
from concourse import bacc, mybir
from concourse.bass_utils import run_bass_kernel_spmd

B = 8          # batch (one element per core)
S = 256        # contraction dim
N = 2048       # feature dim (n and m)
P = 128        # SBUF partitions
KC = S // P    # 2 contraction chunks
MC = N // P    # 16 row chunks of logitsT
H = N // 2     # half width (one PSUM tile)

_CACHE = {}


def _build():
    if "nc" in _CACHE:
        return _CACHE["nc"]

    f32 = mybir.dt.float32
    bf16 = mybir.dt.bfloat16

    nc = bacc.Bacc("TRN2", target_bir_lowering=False, debug=False)
    z1 = nc.dram_tensor("z1", [S, N], bf16, kind="ExternalInput").ap()
    z2 = nc.dram_tensor("z2", [S, N], bf16, kind="ExternalInput").ap()
    # out[:, 0:32]  = negated half maxes  (col 2m+h)
    # out[:, 32:64] = half exp-sums       (col 32+2m+h)
    out_d = nc.dram_tensor("out", [P, 64], f32, kind="ExternalOutput").ap()

    z1r = z1.rearrange("(k p) n -> k p n", p=P)
    z2r = z2.rearrange("(k p) n -> k p n", p=P)

    with tile.TileContext(nc) as tc:
        with (
            tc.tile_pool(name="const", bufs=1) as cpool,
            tc.tile_pool(name="zb", bufs=1) as zpool,
            tc.tile_pool(name="psum", bufs=4, space="PSUM") as ppool,
            tc.tile_pool(name="trash", bufs=3) as tpool,
            tc.tile_pool(name="mx", bufs=8) as mpool,
            tc.tile_pool(name="se", bufs=8) as spool,
        ):
            # ACT exp-table preload, overlapped with the input DMAs.
            dummy = cpool.tile([1, 1], f32, tag="dummy")
            nc.gpsimd.memset(dummy[:], 0.0)
            nc.scalar.activation(
                dummy[:], dummy[:], mybir.ActivationFunctionType.Exp, bias=0.0
            )

            osb = cpool.tile([P, 64], f32, tag="osb")

            z1b = [
                zpool.tile([P, N], bf16, name=f"z1b{k}", tag=f"z1b{k}")
                for k in range(KC)
            ]
            z2b = [
                zpool.tile([P, N], bf16, name=f"z2b{k}", tag=f"z2b{k}")
                for k in range(KC)
            ]
            # Lean DMA plan: first-needed slices first, spread over both
            # DGE-capable queues so dispatch overlaps.
            nc.sync.dma_start(z1b[0][:, 0:512], z1r[0][:, 0:512])
            nc.scalar.dma_start(z1b[1][:, 0:512], z1r[1][:, 0:512])
            nc.sync.dma_start(z2b[0][:, 0:P], z2r[0][:, 0:P])
            nc.scalar.dma_start(z2b[1][:, 0:P], z2r[1][:, 0:P])
            nc.sync.dma_start(z1b[0][:, 512:H], z1r[0][:, 512:H])
            nc.scalar.dma_start(z1b[1][:, 512:H], z1r[1][:, 512:H])
            nc.sync.dma_start(z2b[0][:, P:256], z2r[0][:, P:256])
            nc.scalar.dma_start(z2b[1][:, P:256], z2r[1][:, P:256])
            nc.sync.dma_start(z1b[0][:, H:N], z1r[0][:, H:N])
            nc.scalar.dma_start(z1b[1][:, H:N], z1r[1][:, H:N])
            nc.sync.dma_start(z2b[0][:, 256:N], z2r[0][:, 256:N])
            nc.scalar.dma_start(z2b[1][:, 256:N], z2r[1][:, 256:N])

            # PE warm-up during the input-DMA wait: the HAM clock gate keeps
            # the PE at 1.2 GHz until it has been busy ~3.4us.  A burst of
            # junk matmuls on zeroed SBUF (no DMA dependency) starts that
            # clock early so the first real chunks run at 2.4 GHz.
            warm = cpool.tile([P, 512], bf16, tag="warm")
            nc.gpsimd.memset(warm[:], 0.0)
            jpsum = ppool.tile([P, H], f32, tag="psum")
            for _ in range(6):
                nc.tensor.matmul(
                    jpsum[:, 0:512],
                    lhsT=warm[:, 0:P],
                    rhs=warm[:],
                    start=True,
                    stop=True,
                )

            for m in range(MC):
                ms = slice(m * P, (m + 1) * P)
                for h in range(2):
                    hbase = h * H
                    psum = ppool.tile([P, H], f32, tag="psum")
                    for k in range(KC):
                        for nb in range(2):
                            ns = slice(hbase + nb * 512, hbase + (nb + 1) * 512)
                            nc.tensor.matmul(
                                psum[:, nb * 512 : (nb + 1) * 512],
                                lhsT=z2b[k][:, ms],
                                rhs=z1b[k][:, ns],
                                start=(k == 0),
                                stop=(k == KC - 1),
                            )
                    j = 2 * m + h
                    # negated half-row max -> bias for the exp.  Small
                    # rotating tiles decouple the hot DVE->ACT chain from the
                    # staging tile (GpSimd copies into osb off the hot path).
                    mx_t = mpool.tile([P, 1], f32, tag="mx")
                    se_t = spool.tile([P, 1], f32, tag="se")
                    nc.vector.tensor_reduce(
                        mx_t[:],
                        psum[:],
                        axis=mybir.AxisListType.X,
                        op=mybir.AluOpType.max,
                        negate=True,
                    )
                    # exp(logitsT - halfmax), accumulated along the half row;
                    # exp values written back in place (discarded).
                    nc.scalar.activation(
                        psum[:],
                        psum[:],
                        mybir.ActivationFunctionType.Exp,
                        bias=mx_t[:],
                        scale=1.0,
                        accum_out=se_t[:],
                    )
                    nc.gpsimd.tensor_copy(osb[:, j : j + 1], mx_t[:])
                    nc.gpsimd.tensor_copy(osb[:, 32 + j : 33 + j], se_t[:])
                if m == 13:
                    # Overlap most of the output DMA with the last chunks.
                    nc.sync.dma_start(out_d[:, 0:28], osb[:, 0:28])
                    nc.sync.dma_start(out_d[:, 32:60], osb[:, 32:60])

            nc.sync.dma_start(out_d[:, 28:32], osb[:, 28:32])
            nc.sync.dma_start(out_d[:, 60:64], osb[:, 60:64])

    nc.compile()
    _CACHE["nc"] = nc
    return nc


def _run(z1, z2, **spmd_kwargs):
    """Shard over batch, run on 8 cores, return (loss, BassKernelResults)."""
    nc = _build()
    z1 = np.ascontiguousarray(z1, dtype=np.float32)
    z2 = np.ascontiguousarray(z2, dtype=np.float32)
    in_maps = [
        {
            "z1": np.ascontiguousarray(z1[b].astype(ml_dtypes.bfloat16)),
            "z2": np.ascontiguousarray(z2[b].astype(ml_dtypes.bfloat16)),
        }
        for b in range(B)
    ]
    res = run_bass_kernel_spmd(nc, in_maps, core_ids=list(range(B)), **spmd_kwargs)

    # diag[b, m] = sum_s z1[b,s,m] * z2[b,s,m] in f32 on host (tiny vs the
    # 17 GFLOP device part).
    dg = np.einsum("bsm,bsm->bm", z1, z2, dtype=np.float64)

    total = 0.0
    for b in range(B):
        o = res.results[b]["out"].astype(np.float64)   # [P, 64]
        nmx = o[:, 0:32]                               # [P, 2m+h] negated half max
        se2 = o[:, 32:64]
        ma = -nmx[:, 0::2]                             # [P, MC] left-half max
        mb = -nmx[:, 1::2]
        sea = se2[:, 0::2]
        seb = se2[:, 1::2]
        M = np.maximum(ma, mb)
        se = sea * np.exp(ma - M) + seb * np.exp(mb - M)
        logZ = (M + np.log(se)).T.reshape(N)           # row index = m*P + p
        total += np.sum(dg[b] - logZ)
    loss = -total / (B * N)
    return np.asarray(loss, dtype=np.float32), res


def kernel(z1, z2):
    loss, _ = _run(z1, z2)
    return loss
